# revision 10
# baseline (speedup 1.0000x reference)
"""Trainium2 Bass kernel for nn_CP_TransformerDecoder_Action.

Strategy
--------
Host side (numpy, not timed):
  * The CP adapters and LN affine params are *linear*, so they fold exactly into
    the dense per-layer weights:  Wqkv_eff, Wproj_eff, Wfc1_eff, Wfc2_eff.
  * DP=2 (batch) x TP=4 (heads / hidden) sharding across 8 cores.
  * Weights pre-transposed to matmul lhsT layout, cast to bf16, pre-tiled.
  * Residual stream is kept FEATURE-major (xT [C, tokens]) on device so every
    matmul contracts over the partition dim with zero on-device transposes.

Device (one SPMD program, 8 cores):
  per layer:  LN1 -> qkT/kT (transposed) + v (token-major) -> S^T = k q^T ->
  exp -> mask -> O^T = v_aug^T P^T (ones-column gives softmax denom) ->
  normalize -> proj partial -> bf16 AllReduce(4-core group) -> residual ->
  LN2 -> fc1+gelu -> fc2 partial -> AllReduce -> residual.
  Tokens processed in 2 chunks of 512 so collectives overlap compute.
"""

import numpy as np
import ml_dtypes

L, B, N, C, H, D, R = 4, 2, 1024, 1024, 16, 64, 64
HID = 4 * C
TP = 4                      # tensor-parallel group size
NCORES = 8
CHUNK = 512                 # token chunk (matmul moving free dim)
NCHUNK = N // CHUNK         # 2
KT = C // 128               # 8 C-tiles
HL = H // TP                # 4 heads per core
CL = HL * D                 # 256 local attention features
HIDL = HID // TP            # 1024 local hidden
RG = [[0, 1, 2, 3], [4, 5, 6, 7]]
VS = D + 4                  # v storage stride per head (64 data + 1 ones + pad)

BF16 = ml_dtypes.bfloat16


def _fold_weights(inp):
    """Fold LN affine + CP adapters into dense per-layer weights (fp32 exact)."""
    f32 = np.float32
    u_w = np.asarray(inp['u_w'], f32)       # [R, C]
    v_w = np.asarray(inp['v_w'], f32)       # [C, R]
    cp_c = np.asarray(inp['cp_c'], f32)     # [R, R, R]
    out = []
    for l in range(L):
        g1 = np.asarray(inp['ln1_g'][l], f32); b1 = np.asarray(inp['ln1_b'][l], f32)
        g2 = np.asarray(inp['ln2_g'][l], f32); b2 = np.asarray(inp['ln2_b'][l], f32)
        qkv_w = np.asarray(inp['qkv_w'][l], f32)
        proj_w = np.asarray(inp['proj_w'][l], f32)
        fc1_w = np.asarray(inp['fc1_w'][l], f32)
        fc2_w = np.asarray(inp['fc2_w'][l], f32)
        CPa = np.einsum('abr,rf->abf', cp_c, np.asarray(inp['cp_att'][l], f32))
        CPm = np.einsum('abr,rf->abf', cp_c, np.asarray(inp['mlp_cp'][l], f32))

        Pcat = np.concatenate([CPa[:, :, i] @ v_w.T for i in range(3)], axis=1)   # [R,3C]
        Wqkv_t = (qkv_w * g1[None, :]).T + (u_w * g1[None, :]).T @ Pcat           # [C,3C]
        bqkv = b1 @ qkv_w.T + (b1 @ u_w.T) @ Pcat                                  # [3C]

        Wproj_t = proj_w.T + u_w.T @ (CPa[:, :, 3] @ v_w.T)                        # [C,C]
        bproj = np.asarray(inp['proj_b'][l], f32)

        fc1_cp = CPm[:, :, :4].reshape(R, 4 * R)
        T = np.concatenate([fc1_cp[:, j*R:(j+1)*R] @ v_w.T for j in range(4)], axis=1)
        Wfc1_t = (fc1_w * g2[None, :]).T + (u_w * g2[None, :]).T @ T               # [C,HID]
        bfc1 = np.asarray(inp['fc1_b'][l], f32) + b2 @ fc1_w.T + (b2 @ u_w.T) @ T

        fc2_cp = CPm[:, :, 4:].reshape(R, 4 * R)
        Z = np.concatenate([u_w.T @ fc2_cp[:, j*R:(j+1)*R].T @ v_w.T for j in range(4)], axis=0)
        Wfc2_t = fc2_w.T + Z                                                       # [HID,C]
        bfc2 = np.asarray(inp['fc2_b'][l], f32)
        out.append(dict(Wqkv_t=Wqkv_t, bqkv=bqkv, Wproj_t=Wproj_t, bproj=bproj,
                        Wfc1_t=Wfc1_t, bfc1=bfc1, Wfc2_t=Wfc2_t, bfc2=bfc2))
    return out


def _make_masks():
    """Causal multipliers for diagonal-crossing S^T tiles, rel = key0 - query0."""
    kk = np.arange(128)[:, None]
    qq = np.arange(CHUNK)[None, :]
    return np.stack([(p * 128 + kk) <= qq for p in range(CHUNK // 128)]).astype(BF16)


def build_program(bias_on, gelu_mode="exact", collective_mode="on", loop_mult=1):
    """Build the SPMD Bass/Tile program. bias_on: dict of bools per bias kind.

    gelu_mode="approx" replaces the ACT Gelu table with x*sigmoid(1.702x) so
    the kernel can run under CoreSim (which lacks Gelu); hardware uses "exact".
    """
    from contextlib import ExitStack
    import concourse.mybir as mybir
    import concourse.tile as tile
    from concourse import bacc

    dt = mybir.dt
    AF = mybir.ActivationFunctionType
    nc = bacc.Bacc(num_devices=NCORES)

    xT_p = nc.declare_dram_parameter("xT", [KT, 128, N], dt.float32, isOutput=False)
    wqk_p = nc.declare_dram_parameter("wqk", [L, KT, 128, 2 * CL], dt.bfloat16, isOutput=False)
    wv_p = nc.declare_dram_parameter("wv", [L, KT, 128, CL], dt.bfloat16, isOutput=False)
    wpr_p = nc.declare_dram_parameter("wpr", [L, CL // 128, 128, C], dt.bfloat16, isOutput=False)
    wf1_p = nc.declare_dram_parameter("wf1", [L, KT, 128, HIDL], dt.bfloat16, isOutput=False)
    wf2_p = nc.declare_dram_parameter("wf2", [L, HIDL // 128, 128, C], dt.bfloat16, isOutput=False)
    mask_p = nc.declare_dram_parameter("masks", [4, 128, CHUNK], dt.bfloat16, isOutput=False)
    bias_p = {}
    for nm, shp in (("bqk", [L, 128, 4]), ("bv", [L, 128, CL]),
                    ("bpr", [L, 128, KT]), ("bf1", [L, 128, KT]), ("bf2", [L, 128, KT])):
        if bias_on[nm]:
            bias_p[nm] = nc.declare_dram_parameter(nm, shp, dt.float32, isOutput=False)
    out_p = nc.declare_dram_parameter("out", [KT, 128, N], dt.float32, isOutput=True)

    with tile.TileContext(nc) as tc, ExitStack() as ctx:
        consts = ctx.enter_context(tc.tile_pool(name="consts", bufs=1))
        wpool = ctx.enter_context(tc.tile_pool(name="wpool", bufs=2))
        wfpool = ctx.enter_context(tc.tile_pool(name="wfpool", bufs=1))
        xpool = ctx.enter_context(tc.tile_pool(name="xpool", bufs=1))
        hpool = ctx.enter_context(tc.tile_pool(name="hpool", bufs=2))    # xb/xh, a
        apool = ctx.enter_context(tc.tile_pool(name="apool", bufs=2))    # attn tiles
        espool = ctx.enter_context(tc.tile_pool(name="espool", bufs=3))
        stpool = ctx.enter_context(tc.tile_pool(name="stpool", bufs=4))  # staging
        spool = ctx.enter_context(tc.tile_pool(name="spool", bufs=2))    # small stats
        sqpool = ctx.enter_context(tc.tile_pool(name="sqpool", bufs=3))
        ps_mm = ctx.enter_context(tc.tile_pool(name="ps_mm", bufs=3, space="PSUM"))
        ps_ot = ctx.enter_context(tc.tile_pool(name="ps_ot", bufs=2, space="PSUM"))
        ps_bc = ctx.enter_context(tc.tile_pool(name="ps_bc", bufs=2, space="PSUM"))
        ps_st = ctx.enter_context(tc.tile_pool(name="ps_st", bufs=1, space="PSUM"))
        dram = ctx.enter_context(tc.tile_pool(name="dram", bufs=4, space="DRAM"))

        # ---- constants
        ones_col = consts.tile([128, 1], dt.bfloat16)
        nc.vector.memset(ones_col, 1.0)
        ones_row = consts.tile([1, 128], dt.bfloat16)
        nc.vector.memset(ones_row, 1.0)
        eps_t = consts.tile([1, 1], dt.float32)
        nc.vector.memset(eps_t, 1e-5)
        mask_t = []
        for p in range(4):
            mt_ = consts.tile([128, CHUNK], dt.bfloat16, name=f"mask{p}")
            nc.sync.dma_start(out=mt_, in_=mask_p[p])
            mask_t.append(mt_)

        # ---- residual stream, feature-major fp32
        xt = []
        for k in range(KT):
            t = xpool.tile([128, N], dt.float32, name=f"x{k}")
            nc.sync.dma_start(out=t, in_=xT_p[k])
            xt.append(t)

        def layer_norm(lname, c):
            """Returns list of 8 bf16 tiles xh[k] = normalized x chunk, [128, CHUNK]."""
            ts = slice(c * CHUNK, (c + 1) * CHUNK)
            xb = []
            for k in range(KT):
                t = hpool.tile([128, CHUNK], dt.bfloat16, name=f"xb{k}", tag=f"xb{k}")
                nc.vector.tensor_copy(out=t, in_=xt[k][:, ts])
                xb.append(t)
            stat = ps_st.tile([64, CHUNK], dt.float32, tag="stat")
            for k in range(KT):
                nc.tensor.matmul(stat[0:1, :], ones_col, xb[k],
                                 start=(k == 0), stop=(k == KT - 1))
            for k in range(KT):
                sq = sqpool.tile([128, CHUNK], dt.bfloat16, name="sq", tag="sq")
                nc.vector.tensor_mul(out=sq, in0=xb[k], in1=xb[k])
                nc.tensor.matmul(stat[32:33, :], ones_col, sq,
                                 start=(k == 0), stop=(k == KT - 1))
            s1 = spool.tile([1, CHUNK], dt.float32, tag="s1")           # negmean
            nc.scalar.activation(out=s1, in_=stat[0:1, :], func=AF.Copy,
                                 scale=-1.0 / C)
            s2 = spool.tile([1, CHUNK], dt.float32, tag="s2")           # mean^2
            nc.vector.tensor_mul(out=s2, in0=s1, in1=s1)
            s3 = spool.tile([1, CHUNK], dt.float32, tag="s3")           # msq
            nc.scalar.activation(out=s3, in_=stat[32:33, :], func=AF.Copy,
                                 scale=1.0 / C)
            nc.vector.tensor_sub(out=s3, in0=s3, in1=s2)                # var
            nc.scalar.activation(out=s2, in_=s3, func=AF.Sqrt, bias=eps_t[:, 0:1])
            nc.vector.reciprocal(out=s3, in_=s2)                        # rstd
            nc.vector.tensor_mul(out=s1, in0=s1, in1=s3)                # -mean*rstd
            a_bf = spool.tile([1, CHUNK], dt.bfloat16, tag="a_bf")
            nc.scalar.activation(out=a_bf, in_=s3, func=AF.Copy)
            b_bf = spool.tile([1, CHUNK], dt.bfloat16, tag="b_bf")
            nc.scalar.activation(out=b_bf, in_=s1, func=AF.Copy)
            ps_a = ps_bc.tile([128, CHUNK], dt.float32, tag="bc")
            nc.tensor.matmul(ps_a, ones_row, a_bf, start=True, stop=True)
            a_bc = spool.tile([128, CHUNK], dt.bfloat16, tag="a_bc")
            nc.scalar.activation(out=a_bc, in_=ps_a, func=AF.Copy)
            ps_b = ps_bc.tile([128, CHUNK], dt.float32, tag="bc")
            nc.tensor.matmul(ps_b, ones_row, b_bf, start=True, stop=True)
            b_bc = spool.tile([128, CHUNK], dt.bfloat16, tag="b_bc")
            nc.scalar.activation(out=b_bc, in_=ps_b, func=AF.Copy)
            for k in range(KT):
                nc.vector.tensor_mul(out=xb[k], in0=xb[k], in1=a_bc)
                nc.vector.tensor_add(out=xb[k], in0=xb[k], in1=b_bc)
            return xb

        def all_reduce(stage_wide, lname):
            """AllReduce one wide [128, KT*CHUNK] bf16 staging tile.

            One staging DMA on the gpsimd (Pool) queue so neither the SP queue
            (weight prefetch) nor compute queues block behind the collective."""
            arin = dram.tile([128, KT * CHUNK], dt.bfloat16, name="arin", tag="arin")
            arout = dram.tile([128, KT * CHUNK], dt.bfloat16, name="arout", tag="arout")
            nc.gpsimd.dma_start(out=arin, in_=stage_wide)
            if collective_mode == "on":
                nc.gpsimd.collective_compute(
                    "AllReduce", mybir.AluOpType.add, replica_groups=RG,
                    ins=[arin.opt()], outs=[arout.opt()])
            else:  # timing ablation: local copy instead of AllReduce (wrong math)
                nc.gpsimd.dma_start(out=arout.opt(), in_=arin.opt())
            return arout

        def resid_update(arout, c, tag):
            """Load AllReduce result (one wide DMA on SP) and add into residual."""
            ts = slice(c * CHUNK, (c + 1) * CHUNK)
            up = stpool.tile([128, KT * CHUNK], dt.bfloat16, name=f"upd{tag}",
                             tag="upd", bufs=2)
            nc.sync.dma_start(out=up, in_=arout)
            for mt in range(KT):
                nc.vector.tensor_add(out=xt[mt][:, ts], in0=xt[mt][:, ts],
                                     in1=up[:, mt*CHUNK:(mt+1)*CHUNK])

        def load_weights(l):
            """Weight DMAs on the SP queue (pure prefetch, no collective deps)."""
            W = {}
            wqk = []
            for k in range(KT):
                t = wpool.tile([128, 2 * CL], dt.bfloat16, name=f"wqk{k}", tag=f"wqk{k}")
                nc.sync.dma_start(out=t, in_=wqk_p[l, k])
                wqk.append(t)
            wv = []
            for k in range(KT):
                t = wpool.tile([128, CL], dt.bfloat16, name=f"wv{k}", tag=f"wv{k}", bufs=1)
                nc.sync.dma_start(out=t, in_=wv_p[l, k])
                wv.append(t)
            wpr = []
            for j in range(CL // 128):
                t = wpool.tile([128, C], dt.bfloat16, name=f"wpr{j}", tag=f"wpr{j}", bufs=1)
                nc.sync.dma_start(out=t, in_=wpr_p[l, j])
                wpr.append(t)
            wf1 = []
            for k in range(KT):
                t = wfpool.tile([128, HIDL], dt.bfloat16, name=f"wf1{k}", tag=f"wf1{k}")
                nc.sync.dma_start(out=t, in_=wf1_p[l, k])
                wf1.append(t)
            wf2 = []
            for k in range(HIDL // 128):
                t = wfpool.tile([128, C], dt.bfloat16, name=f"wf2{k}", tag=f"wf2{k}")
                nc.sync.dma_start(out=t, in_=wf2_p[l, k])
                wf2.append(t)
            bias_t = {}
            for nm in bias_p:
                t = wpool.tile(list(bias_p[nm].shape[1:]), dt.float32,
                               name=f"{nm}t", tag=f"{nm}t")
                nc.sync.dma_start(out=t, in_=bias_p[nm][l])
                bias_t[nm] = t
            W.update(wqk=wqk, wv=wv, wpr=wpr, wf1=wf1, wf2=wf2, bias=bias_t)
            return W

        def qkv_block(W, c, xh, kt_store, v_store):
            """qT/kT (feature-major, 2 heads per [128,CHUNK] tile) + token-major v."""
            bias_t = W['bias']
            qt2 = []
            for mt in range(2 * CL // 128):   # 4 Mtiles: q q k k
                ps = ps_mm.tile([128, CHUNK], dt.float32, tag="mm")
                for k in range(KT):
                    nc.tensor.matmul(ps, W['wqk'][k][:, mt*128:(mt+1)*128], xh[k],
                                     start=(k == 0), stop=(k == KT - 1))
                j = mt % 2
                if mt < 2:
                    dst = apool.tile([128, CHUNK], dt.bfloat16,
                                     name=f"qT{j}", tag=f"qT{j}")
                    qt2.append(dst)
                else:
                    dst = apool.tile([128, CHUNK], dt.bfloat16,
                                     name=f"kT{c}_{j}", tag=f"kT{c}_{j}", bufs=1)
                    kt_store[(c, j)] = dst
                if bias_on["bqk"]:
                    nc.scalar.activation(out=dst, in_=ps, func=AF.Identity,
                                         bias=bias_t["bqk"][:, mt:mt+1])
                else:
                    nc.scalar.activation(out=dst, in_=ps, func=AF.Copy)

            for mt in range(CHUNK // 128):
                ps = ps_mm.tile([128, CL], dt.float32, tag="mm")
                for k in range(KT):
                    nc.tensor.matmul(ps, xh[k][:, mt*128:(mt+1)*128], W['wv'][k],
                                     start=(k == 0), stop=(k == KT - 1))
                kti = c * (CHUNK // 128) + mt
                vt = apool.tile([128, HL * VS], dt.bfloat16,
                                name=f"v{kti}", tag=f"v{kti}", bufs=1)
                vv = vt.rearrange("p (h e) -> p h e", e=VS)
                nc.scalar.activation(out=vv[:, :, 0:D],
                                     in_=ps.rearrange("p (h e) -> p h e", e=D),
                                     func=AF.Copy)
                nc.vector.memset(vv[:, :, D:D+1], 1.0)
                if bias_on["bv"]:
                    nc.vector.tensor_add(
                        out=vv[:, :, 0:D], in0=vv[:, :, 0:D],
                        in1=bias_t["bv"].rearrange("p (h e) -> p h e", e=D))
                v_store[kti] = vt
            return qt2

        def attn_block(c, qt2, kt_store, v_store):
            """Per head: S^T -> exp -> mask -> O^T (ones column gives denom)."""
            ot_sb = [apool.tile([128, CHUNK], dt.bfloat16, name=f"oT{j}", tag=f"oT{j}")
                     for j in range(CL // 128)]
            nkt = (c + 1) * (CHUNK // 128)
            for h in range(HL):
                j, half = h // 2, h % 2
                pr = slice(half * 64, half * 64 + 64)
                ot_ps = ps_ot.tile([65, CHUNK], dt.float32, tag="ot")
                for kt in range(nkt):
                    kc, km = kt // (CHUNK // 128), kt % (CHUNK // 128)
                    s_ps = ps_mm.tile([128, CHUNK], dt.float32, tag="mm")
                    nc.tensor.matmul(s_ps,
                                     kt_store[(kc, j)][pr, km*128:(km+1)*128],
                                     qt2[j][pr, :], start=True, stop=True)
                    es = espool.tile([128, CHUNK], dt.bfloat16, name="es", tag="es")
                    nc.scalar.activation(out=es, in_=s_ps, func=AF.Exp)
                    rel = kt * 128 - c * CHUNK
                    if rel >= 0:
                        nc.vector.tensor_mul(out=es, in0=es, in1=mask_t[rel // 128])
                    nc.tensor.matmul(ot_ps,
                                     v_store[kt][:, h*VS:h*VS+D+1], es,
                                     start=(kt == 0), stop=(kt == nkt - 1))
                recip = spool.tile([1, CHUNK], dt.float32, tag="recip")
                nc.vector.reciprocal(out=recip, in_=ot_ps[64:65, :])
                recb = spool.tile([1, CHUNK], dt.bfloat16, tag="recb")
                nc.scalar.activation(out=recb, in_=recip, func=AF.Copy)
                rb_ps = ps_bc.tile([128, CHUNK], dt.float32, tag="bc")
                nc.tensor.matmul(rb_ps[0:64, :], ones_row[:, 0:64], recb,
                                 start=True, stop=True)
                rb_sb = spool.tile([64, CHUNK], dt.bfloat16, tag="rb_sb")
                nc.scalar.activation(out=rb_sb, in_=rb_ps[0:64, :], func=AF.Copy)
                nc.vector.tensor_mul(out=ot_sb[j][half*64:(half+1)*64, :],
                                     in0=ot_ps[0:64, :], in1=rb_sb)
            return ot_sb

        def proj_block(W, c, ot_sb):
            bias_t = W['bias']
            stw = stpool.tile([128, KT * CHUNK], dt.bfloat16, name="prst",
                              tag="stage", bufs=2)
            for mt in range(KT):
                ps = ps_mm.tile([128, CHUNK], dt.float32, tag="mm")
                for j in range(CL // 128):
                    nc.tensor.matmul(ps, W['wpr'][j][:, mt*128:(mt+1)*128], ot_sb[j],
                                     start=(j == 0), stop=(j == CL // 128 - 1))
                st = stw[:, mt*CHUNK:(mt+1)*CHUNK]
                if bias_on["bpr"]:
                    nc.scalar.activation(out=st, in_=ps, func=AF.Identity,
                                         bias=bias_t["bpr"][:, mt:mt+1])
                else:
                    nc.scalar.activation(out=st, in_=ps, func=AF.Copy)
            return stw

        def ffn_block(W, c, xh2):
            bias_t = W['bias']
            a_sb = []
            for mt in range(HIDL // 128):
                ps = ps_mm.tile([128, CHUNK], dt.float32, tag="mm")
                for k in range(KT):
                    nc.tensor.matmul(ps, W['wf1'][k][:, mt*128:(mt+1)*128], xh2[k],
                                     start=(k == 0), stop=(k == KT - 1))
                at = hpool.tile([128, CHUNK], dt.bfloat16, name=f"ga{mt}",
                                tag=f"ga{mt}", bufs=1)
                if gelu_mode == "exact":
                    if bias_on["bf1"]:
                        nc.scalar.activation(out=at, in_=ps, func=AF.Gelu,
                                             bias=bias_t["bf1"][:, mt:mt+1])
                    else:
                        nc.scalar.activation(out=at, in_=ps, func=AF.Gelu)
                else:
                    assert not bias_on["bf1"]
                    sg = sqpool.tile([128, CHUNK], dt.bfloat16, name="sg", tag="sq")
                    nc.scalar.activation(out=sg, in_=ps, func=AF.Sigmoid,
                                         scale=1.702)
                    nc.vector.tensor_mul(out=at, in0=sg, in1=ps)
                a_sb.append(at)
            stw = stpool.tile([128, KT * CHUNK], dt.bfloat16, name="f2st",
                              tag="stage", bufs=2)
            for mt in range(KT):
                ps = ps_mm.tile([128, CHUNK], dt.float32, tag="mm")
                for k in range(HIDL // 128):
                    nc.tensor.matmul(ps, W['wf2'][k][:, mt*128:(mt+1)*128], a_sb[k],
                                     start=(k == 0), stop=(k == HIDL // 128 - 1))
                st = stw[:, mt*CHUNK:(mt+1)*CHUNK]
                if bias_on["bf2"]:
                    nc.scalar.activation(out=st, in_=ps, func=AF.Identity,
                                         bias=bias_t["bf2"][:, mt:mt+1])
                else:
                    nc.scalar.activation(out=st, in_=ps, func=AF.Copy)
            return stw

        # Two-chunk software pipeline: each AllReduce is issued before ~80us of
        # independent compute from the other chunk, so its latency (plus the
        # serial residual->LN chain behind it) hides completely.
        pending = [None] * NCHUNK      # previous layer's fc2 AllReduces
        for li in range(L * loop_mult):
            l = li % L
            W = load_weights(l)
            kt_store, v_store = {}, {}
            ar_pr = [None] * NCHUNK
            for c in range(NCHUNK):
                if pending[c] is not None:
                    resid_update(pending[c], c, f"f2p{c}")
                    pending[c] = None
                xh = layer_norm(f"ln1_{l}_{c}", c)
                qt2 = qkv_block(W, c, xh, kt_store, v_store)
                ot_sb = attn_block(c, qt2, kt_store, v_store)
                stage = proj_block(W, c, ot_sb)
                ar_pr[c] = all_reduce(stage, f"pr_{l}_{c}")
            for c in range(NCHUNK):
                resid_update(ar_pr[c], c, f"pr{c}")
                xh2 = layer_norm(f"ln2_{l}_{c}", c)
                stage2 = ffn_block(W, c, xh2)
                pending[c] = all_reduce(stage2, f"f2_{l}_{c}")
        for c in range(NCHUNK):
            resid_update(pending[c], c, f"fin{c}")

        for k in range(KT):
            nc.sync.dma_start(out=out_p[k], in_=xt[k])

    if not nc.is_finalized():
        nc.finalize()
    return nc


def _prep_core_inputs(inputs, folded):
    """Per-core in_maps (host-side sharding + layout + bf16 cast)."""
    x = np.asarray(inputs['x'], np.float32)
    masks = _make_masks()
    scale = np.float32(D ** -0.5)

    per_core = []
    bias_on = {k: False for k in ("bqk", "bv", "bpr", "bf1", "bf2")}
    shard_cache = {}
    for cid in range(NCORES):
        r, b = cid % TP, cid // TP
        if r not in shard_cache:
            wqk, wv, wpr, wf1, wf2 = [], [], [], [], []
            bqk, bv, bpr, bf1, bf2 = [], [], [], [], []
            for l in range(L):
                F = folded[l]
                Wq = F['Wqkv_t'][:, r*CL:(r+1)*CL] * scale
                Wk = F['Wqkv_t'][:, C + r*CL: C + (r+1)*CL]
                Wv = F['Wqkv_t'][:, 2*C + r*CL: 2*C + (r+1)*CL]
                wqk.append(np.concatenate([Wq, Wk], axis=1).reshape(KT, 128, 2*CL))
                wv.append(Wv.reshape(KT, 128, CL))
                wpr.append(F['Wproj_t'][r*CL:(r+1)*CL, :].reshape(CL//128, 128, C))
                wf1.append(F['Wfc1_t'][:, r*HIDL:(r+1)*HIDL].reshape(KT, 128, HIDL))
                wf2.append(F['Wfc2_t'][r*HIDL:(r+1)*HIDL, :].reshape(HIDL//128, 128, C))
                bq = F['bqkv'][r*CL:(r+1)*CL] * scale
                bk = F['bqkv'][C + r*CL: C + (r+1)*CL]
                bqk.append(np.concatenate([bq, bk]).reshape(4, 128).T)
                bv.append(np.broadcast_to(
                    F['bqkv'][2*C + r*CL: 2*C + (r+1)*CL], (128, CL)).copy())
                bpr.append(F['bproj'].reshape(KT, 128).T / TP)
                bf1.append(F['bfc1'][r*HIDL:(r+1)*HIDL].reshape(KT, 128).T)
                bf2.append(F['bfc2'].reshape(KT, 128).T / TP)
            shard = dict(
                wqk=np.stack(wqk).astype(BF16), wv=np.stack(wv).astype(BF16),
                wpr=np.stack(wpr).astype(BF16), wf1=np.stack(wf1).astype(BF16),
                wf2=np.stack(wf2).astype(BF16),
                bqk=np.stack(bqk).astype(np.float32), bv=np.stack(bv).astype(np.float32),
                bpr=np.stack(bpr).astype(np.float32), bf1=np.stack(bf1).astype(np.float32),
                bf2=np.stack(bf2).astype(np.float32))
            shard_cache[r] = shard
        shard = shard_cache[r]
        m = dict(shard)
        m['xT'] = np.ascontiguousarray(x[b].T).reshape(KT, 128, N)
        m['masks'] = masks
        per_core.append(m)

    for nm in bias_on:
        bias_on[nm] = any(bool(np.abs(m[nm]).max() > 0) for m in per_core)
    for m in per_core:
        for nm in list(m):
            if nm in bias_on and not bias_on[nm]:
                del m[nm]
    return per_core, bias_on


LAST_RESULT = None


def kernel(**inputs):
    global LAST_RESULT
    from concourse.bass_utils import run_bass_kernel_spmd
    folded = _fold_weights(inputs)
    in_maps, bias_on = _prep_core_inputs(inputs, folded)
    nc = build_program(bias_on)
    res = run_bass_kernel_spmd(nc, in_maps, core_ids=list(range(NCORES)))
    LAST_RESULT = res
    outs = []
    for b in range(B):
        o = res.results[b * TP]["out"].reshape(C, N).T    # [tokens, C]
        outs.append(o)
    return np.stack(outs).astype(np.float32)


if __name__ == "__main__":
    import reference
    inp = reference.setup_inputs()
    out = kernel(**{k: np.asarray(v) for k, v in inp.items()})
    exp = np.asarray(reference.reference(**inp))
    err = np.abs(out - exp).max() / np.abs(exp).max()
    print("Relative error:", err)



# revision 49
# speedup vs baseline: 5.9574x; 5.9574x over previous
"""Trainium2 Bass kernel for nn_CP_TransformerDecoder_Action.

Strategy
--------
Host side (numpy, not timed):
  * The CP adapters and LN affine params are *linear*, so they fold exactly into
    the dense per-layer weights:  Wqkv_eff, Wproj_eff, Wfc1_eff, Wfc2_eff.
  * DP=2 (batch) x TP=4 (heads / hidden) sharding across 8 cores.
  * Weights pre-transposed to matmul lhsT layout, cast to bf16, pre-tiled.
  * Residual stream is kept FEATURE-major (xT [C, tokens]) on device so every
    matmul contracts over the partition dim with zero on-device transposes.

Device (one SPMD program, 8 cores):
  per layer:  LN1 -> qkT/kT (transposed) + v (token-major) -> S^T = k q^T ->
  exp -> mask -> O^T = v_aug^T P^T (ones-column gives softmax denom) ->
  normalize -> proj partial -> bf16 AllReduce(4-core group) -> residual ->
  LN2 -> fc1+gelu -> fc2 partial -> AllReduce -> residual.

  Tokens are processed in 2 chunks of 512 arranged as a software pipeline:
  each AllReduce is issued before ~80us of independent compute from the other
  chunk, hiding collective latency and the serial residual->LN chains.
  AllReduce staging uses one wide [128,4096] tile and a single DMA per
  direction (staging on the gpsimd queue, result fetch on SP, so weight
  prefetch never queues behind a collective). The residual stream is bf16
  (tolerance is 2e-2); the residual add is fused into the LN's bf16 working
  copy, with the bf16 commit emitted off the critical path. rstd uses
  exp(-0.5*ln(var+eps)) with a dummy [1,1] Ln prefetching the ACT table while
  the AllReduce is in flight.
"""

import numpy as np
import ml_dtypes

L, B, N, C, H, D, R = 4, 2, 1024, 1024, 16, 64, 64
HID = 4 * C
TP = 4                      # tensor-parallel group size
NCORES = 8
CHUNK = 512                 # token chunk (matmul moving free dim)
NCHUNK = N // CHUNK         # 2
KT = C // 128               # 8 C-tiles
HL = H // TP                # 4 heads per core
CL = HL * D                 # 256 local attention features
HIDL = HID // TP            # 1024 local hidden
RG = [[0, 1, 2, 3], [4, 5, 6, 7]]
VS = D + 4                  # v storage stride per head (64 data + 1 ones + pad)

BF16 = ml_dtypes.bfloat16


def _fold_weights(inp):
    """Fold LN affine + CP adapters into dense per-layer weights (fp32 exact)."""
    f32 = np.float32
    u_w = np.asarray(inp['u_w'], f32)       # [R, C]
    v_w = np.asarray(inp['v_w'], f32)       # [C, R]
    cp_c = np.asarray(inp['cp_c'], f32)     # [R, R, R]
    out = []
    for l in range(L):
        g1 = np.asarray(inp['ln1_g'][l], f32); b1 = np.asarray(inp['ln1_b'][l], f32)
        g2 = np.asarray(inp['ln2_g'][l], f32); b2 = np.asarray(inp['ln2_b'][l], f32)
        qkv_w = np.asarray(inp['qkv_w'][l], f32)
        proj_w = np.asarray(inp['proj_w'][l], f32)
        fc1_w = np.asarray(inp['fc1_w'][l], f32)
        fc2_w = np.asarray(inp['fc2_w'][l], f32)
        CPa = np.einsum('abr,rf->abf', cp_c, np.asarray(inp['cp_att'][l], f32))
        CPm = np.einsum('abr,rf->abf', cp_c, np.asarray(inp['mlp_cp'][l], f32))

        Pcat = np.concatenate([CPa[:, :, i] @ v_w.T for i in range(3)], axis=1)   # [R,3C]
        Wqkv_t = (qkv_w * g1[None, :]).T + (u_w * g1[None, :]).T @ Pcat           # [C,3C]
        bqkv = b1 @ qkv_w.T + (b1 @ u_w.T) @ Pcat                                  # [3C]

        Wproj_t = proj_w.T + u_w.T @ (CPa[:, :, 3] @ v_w.T)                        # [C,C]
        bproj = np.asarray(inp['proj_b'][l], f32)

        fc1_cp = CPm[:, :, :4].reshape(R, 4 * R)
        T = np.concatenate([fc1_cp[:, j*R:(j+1)*R] @ v_w.T for j in range(4)], axis=1)
        Wfc1_t = (fc1_w * g2[None, :]).T + (u_w * g2[None, :]).T @ T               # [C,HID]
        bfc1 = np.asarray(inp['fc1_b'][l], f32) + b2 @ fc1_w.T + (b2 @ u_w.T) @ T

        fc2_cp = CPm[:, :, 4:].reshape(R, 4 * R)
        Z = np.concatenate([u_w.T @ fc2_cp[:, j*R:(j+1)*R].T @ v_w.T for j in range(4)], axis=0)
        Wfc2_t = fc2_w.T + Z                                                       # [HID,C]
        bfc2 = np.asarray(inp['fc2_b'][l], f32)
        out.append(dict(Wqkv_t=Wqkv_t, bqkv=bqkv, Wproj_t=Wproj_t, bproj=bproj,
                        Wfc1_t=Wfc1_t, bfc1=bfc1, Wfc2_t=Wfc2_t, bfc2=bfc2))
    return out


def _make_masks():
    """Causal multipliers for diagonal-crossing S^T tiles, rel = key0 - query0."""
    kk = np.arange(128)[:, None]
    qq = np.arange(CHUNK)[None, :]
    return np.stack([(p * 128 + kk) <= qq for p in range(CHUNK // 128)]).astype(BF16)


def build_program(bias_on, gelu_mode="exact", collective_mode="on", loop_mult=1,
                  pool_commit=False, pool_bcast=False, interleave=False):
    """Build the SPMD Bass/Tile program. bias_on: dict of bools per bias kind.

    gelu_mode="approx" replaces the ACT Gelu table with x*sigmoid(1.702x) so
    the kernel can run under CoreSim (which lacks Gelu); hardware uses "exact".
    """
    from contextlib import ExitStack
    import concourse.mybir as mybir
    import concourse.tile as tile
    from concourse import bacc

    dt = mybir.dt
    AF = mybir.ActivationFunctionType
    nc = bacc.Bacc(num_devices=NCORES)

    xT_p = nc.declare_dram_parameter("xT", [KT, 128, N], dt.bfloat16, isOutput=False)
    wqk_p = nc.declare_dram_parameter("wqk", [L, KT, 128, 2 * CL], dt.bfloat16, isOutput=False)
    wv_p = nc.declare_dram_parameter("wv", [L, KT, 128, CL], dt.bfloat16, isOutput=False)
    wpr_p = nc.declare_dram_parameter("wpr", [L, CL // 128, 128, C], dt.bfloat16, isOutput=False)
    wf1_p = nc.declare_dram_parameter("wf1", [L, KT, 128, HIDL], dt.bfloat16, isOutput=False)
    wf2_p = nc.declare_dram_parameter("wf2", [L, HIDL // 128, 128, C], dt.bfloat16, isOutput=False)
    mask_p = nc.declare_dram_parameter("masks", [4, 128, CHUNK], dt.bfloat16, isOutput=False)
    bias_p = {}
    for nm, shp in (("bqk", [L, 128, 4]), ("bv", [L, 128, CL]),
                    ("bpr", [L, 128, KT]), ("bf1", [L, 128, KT]), ("bf2", [L, 128, KT])):
        if bias_on[nm]:
            bias_p[nm] = nc.declare_dram_parameter(nm, shp, dt.float32, isOutput=False)
    out_p = nc.declare_dram_parameter("out", [KT, 128, N], dt.float32, isOutput=True)

    with tile.TileContext(nc) as tc, ExitStack() as ctx:
        consts = ctx.enter_context(tc.tile_pool(name="consts", bufs=1))
        wpool = ctx.enter_context(tc.tile_pool(name="wpool", bufs=2))
        wfpool = ctx.enter_context(tc.tile_pool(name="wfpool", bufs=1))
        xpool = ctx.enter_context(tc.tile_pool(name="xpool", bufs=1))
        hpool = ctx.enter_context(tc.tile_pool(name="hpool", bufs=2))    # xb/xh, a
        apool = ctx.enter_context(tc.tile_pool(name="apool", bufs=2))    # attn tiles
        espool = ctx.enter_context(tc.tile_pool(name="espool", bufs=3))
        stpool = ctx.enter_context(tc.tile_pool(name="stpool", bufs=4))  # staging
        spool = ctx.enter_context(tc.tile_pool(name="spool", bufs=2))    # small stats
        sqpool = ctx.enter_context(tc.tile_pool(name="sqpool", bufs=3))
        ps_mm = ctx.enter_context(tc.tile_pool(name="ps_mm", bufs=3, space="PSUM"))
        ps_ot = ctx.enter_context(tc.tile_pool(name="ps_ot", bufs=2, space="PSUM"))
        ps_bc = ctx.enter_context(tc.tile_pool(name="ps_bc", bufs=2, space="PSUM"))
        ps_st = ctx.enter_context(tc.tile_pool(name="ps_st", bufs=1, space="PSUM"))
        dram = ctx.enter_context(tc.tile_pool(name="dram", bufs=4, space="DRAM"))

        # ---- constants
        ones_col = consts.tile([128, 1], dt.bfloat16)
        nc.vector.memset(ones_col, 1.0)
        ones_row = consts.tile([1, 128], dt.bfloat16)
        nc.vector.memset(ones_row, 1.0)
        mones_row = consts.tile([1, 128], dt.bfloat16)
        nc.vector.memset(mones_row, -1.0)
        eps_t = consts.tile([1, 1], dt.float32)
        nc.vector.memset(eps_t, 1e-5)
        mask_t = []
        for p in range(4):
            mt_ = consts.tile([128, CHUNK], dt.bfloat16, name=f"mask{p}")
            nc.sync.dma_start(out=mt_, in_=mask_p[p])
            mask_t.append(mt_)

        # ---- residual stream, feature-major bf16 (fp32 not needed at 2e-2 tol)
        xt = []
        for k in range(KT):
            t = xpool.tile([128, N], dt.bfloat16, name=f"x{k}")
            nc.sync.dma_start(out=t, in_=xT_p[k])
            xt.append(t)

        def layer_norm(lname, c, up=None, nxt=AF.Exp):
            """Returns list of 8 bf16 tiles xh[k] = normalized x chunk, [128, CHUNK].

            If `up` (a wide [128, KT*CHUNK] bf16 AllReduce result) is given, the
            residual add is fused into the bf16 working copy (critical path) and
            the commit into xt is deferred off the critical path.
            Dummy [1,1] activations prefetch the sqrt/exp ACT tables while the
            engine is otherwise idle, so no table load sits on the serial
            stats->rstd->apply chain."""
            ts = slice(c * CHUNK, (c + 1) * CHUNK)
            dum = spool.tile([1, 1], dt.float32, tag="dum")
            nc.scalar.activation(out=dum, in_=eps_t, func=AF.Ln)
            xb = []
            for k in range(KT):
                t = hpool.tile([128, CHUNK], dt.bfloat16, name=f"xb{k}", tag=f"xb{k}")
                if up is None:
                    nc.vector.tensor_copy(out=t, in_=xt[k][:, ts])
                else:
                    nc.vector.tensor_add(out=t, in0=xt[k][:, ts],
                                         in1=up[:, k*CHUNK:(k+1)*CHUNK])
                xb.append(t)
            stat = ps_st.tile([64, CHUNK], dt.float32, tag="stat")
            for k in range(KT):
                nc.tensor.matmul(stat[0:1, :], ones_col, xb[k],
                                 start=(k == 0), stop=(k == KT - 1))
            for k in range(KT):
                sq = sqpool.tile([128, CHUNK], dt.bfloat16, name="sq", tag="sq")
                nc.vector.tensor_mul(out=sq, in0=xb[k], in1=xb[k])
                nc.tensor.matmul(stat[32:33, :], ones_col, sq,
                                 start=(k == 0), stop=(k == KT - 1))
            if up is not None:   # deferred fp32 residual commit, off critical path
                eng = nc.gpsimd if pool_commit else nc.vector
                for k in range(KT):
                    eng.tensor_add(out=xt[k][:, ts], in0=xt[k][:, ts],
                                   in1=up[:, k*CHUNK:(k+1)*CHUNK])
            sm = spool.tile([1, CHUNK], dt.float32, tag="sm")    # mean
            nc.scalar.activation(out=sm, in_=stat[0:1, :], func=AF.Copy,
                                 scale=1.0 / C)
            msq = spool.tile([1, CHUNK], dt.float32, tag="msq")  # meansq
            nc.scalar.activation(out=msq, in_=stat[32:33, :], func=AF.Copy,
                                 scale=1.0 / C)
            m2 = spool.tile([1, CHUNK], dt.float32, tag="m2")
            nc.vector.tensor_mul(out=m2, in0=sm, in1=sm)
            var = spool.tile([1, CHUNK], dt.float32, tag="var")
            nc.vector.tensor_sub(out=var, in0=msq, in1=m2)
            lnv = spool.tile([1, CHUNK], dt.float32, tag="lnv")
            nc.scalar.activation(out=lnv, in_=var, func=AF.Ln, bias=eps_t[:, 0:1])
            a_bf = spool.tile([1, CHUNK], dt.bfloat16, tag="a_bf")   # rstd
            nc.scalar.activation(out=a_bf, in_=lnv, func=AF.Exp, scale=-0.5)
            b_bf = spool.tile([1, CHUNK], dt.bfloat16, tag="b_bf")   # mean*rstd
            nc.vector.tensor_mul(out=b_bf, in0=sm, in1=a_bf)
            ps_a = ps_bc.tile([128, CHUNK], dt.float32, tag="bc")
            nc.tensor.matmul(ps_a, ones_row, a_bf, start=True, stop=True)
            a_bc = spool.tile([128, CHUNK], dt.bfloat16, tag="a_bc")
            nc.scalar.activation(out=a_bc, in_=ps_a, func=AF.Copy)
            ps_b = ps_bc.tile([128, CHUNK], dt.float32, tag="bc")
            nc.tensor.matmul(ps_b, mones_row, b_bf, start=True, stop=True)
            b_bc = spool.tile([128, CHUNK], dt.bfloat16, tag="b_bc")
            nc.scalar.activation(out=b_bc, in_=ps_b, func=AF.Copy)
            for k in range(KT):
                nc.vector.tensor_mul(out=xb[k], in0=xb[k], in1=a_bc)
                nc.vector.tensor_add(out=xb[k], in0=xb[k], in1=b_bc)
            return xb

        def all_reduce(stage_wide, lname):
            """AllReduce one wide [128, KT*CHUNK] bf16 staging tile.

            One staging DMA on the gpsimd (Pool) queue so neither the SP queue
            (weight prefetch) nor compute queues block behind the collective."""
            arin = dram.tile([128, KT * CHUNK], dt.bfloat16, name="arin", tag="arin")
            arout = dram.tile([128, KT * CHUNK], dt.bfloat16, name="arout", tag="arout")
            nc.gpsimd.dma_start(out=arin, in_=stage_wide)
            if collective_mode == "on":
                nc.gpsimd.collective_compute(
                    "AllReduce", mybir.AluOpType.add, replica_groups=RG,
                    ins=[arin.opt()], outs=[arout.opt()])
            else:  # timing ablation: local copy instead of AllReduce (wrong math)
                nc.gpsimd.dma_start(out=arout.opt(), in_=arin.opt())
            return arout

        def load_upd(arout, tag):
            """Fetch an AllReduce result with one wide DMA on the SP queue."""
            up = stpool.tile([128, KT * CHUNK], dt.bfloat16, name=f"upd{tag}",
                             tag="upd", bufs=2)
            nc.sync.dma_start(out=up, in_=arout)
            return up

        def resid_update(arout, c, tag):
            """Epilogue-only: load AllReduce result and add into the residual."""
            ts = slice(c * CHUNK, (c + 1) * CHUNK)
            up = load_upd(arout, tag)
            for mt in range(KT):
                nc.vector.tensor_add(out=xt[mt][:, ts], in0=xt[mt][:, ts],
                                     in1=up[:, mt*CHUNK:(mt+1)*CHUNK])

        def load_weights(l):
            """Weight DMAs on the SP queue (pure prefetch, no collective deps)."""
            W = {}
            wqk = []
            for k in range(KT):
                t = wpool.tile([128, 2 * CL], dt.bfloat16, name=f"wqk{k}", tag=f"wqk{k}")
                nc.sync.dma_start(out=t, in_=wqk_p[l, k])
                wqk.append(t)
            wv = []
            for k in range(KT):
                t = wpool.tile([128, CL], dt.bfloat16, name=f"wv{k}", tag=f"wv{k}", bufs=1)
                nc.sync.dma_start(out=t, in_=wv_p[l, k])
                wv.append(t)
            wpr = []
            for j in range(CL // 128):
                t = wpool.tile([128, C], dt.bfloat16, name=f"wpr{j}", tag=f"wpr{j}", bufs=1)
                nc.sync.dma_start(out=t, in_=wpr_p[l, j])
                wpr.append(t)
            wf1 = []
            for k in range(KT):
                t = wfpool.tile([128, HIDL], dt.bfloat16, name=f"wf1{k}", tag=f"wf1{k}")
                nc.sync.dma_start(out=t, in_=wf1_p[l, k])
                wf1.append(t)
            wf2 = []
            for k in range(HIDL // 128):
                t = wfpool.tile([128, C], dt.bfloat16, name=f"wf2{k}", tag=f"wf2{k}")
                nc.sync.dma_start(out=t, in_=wf2_p[l, k])
                wf2.append(t)
            bias_t = {}
            for nm in bias_p:
                t = wpool.tile(list(bias_p[nm].shape[1:]), dt.float32,
                               name=f"{nm}t", tag=f"{nm}t")
                nc.sync.dma_start(out=t, in_=bias_p[nm][l])
                bias_t[nm] = t
            W.update(wqk=wqk, wv=wv, wpr=wpr, wf1=wf1, wf2=wf2, bias=bias_t)
            return W

        def qkv_gen(W, c, xh, kt_store, v_store, qt2_out):
            """Generator: qT/kT (2 heads per [128,CHUNK] tile) + token-major v.

            Yields after each matmul group so the caller can interleave the
            emission with another chunk's attention (fills PE while ACT does
            that chunk's exps)."""
            bias_t = W['bias']
            for mt in range(2 * CL // 128):   # 4 Mtiles: q q k k
                ps = ps_mm.tile([128, CHUNK], dt.float32, tag="mm")
                for k in range(KT):
                    nc.tensor.matmul(ps, W['wqk'][k][:, mt*128:(mt+1)*128], xh[k],
                                     start=(k == 0), stop=(k == KT - 1))
                j = mt % 2
                if mt < 2:
                    dst = apool.tile([128, CHUNK], dt.bfloat16,
                                     name=f"qT{j}", tag=f"qT{j}")
                    qt2_out.append(dst)
                else:
                    dst = apool.tile([128, CHUNK], dt.bfloat16,
                                     name=f"kT{c}_{j}", tag=f"kT{c}_{j}", bufs=1)
                    kt_store[(c, j)] = dst
                if bias_on["bqk"]:
                    nc.scalar.activation(out=dst, in_=ps, func=AF.Identity,
                                         bias=bias_t["bqk"][:, mt:mt+1])
                else:
                    nc.scalar.activation(out=dst, in_=ps, func=AF.Copy)
                yield

            for mt in range(CHUNK // 128):
                ps = ps_mm.tile([128, CL], dt.float32, tag="mm")
                for k in range(KT):
                    nc.tensor.matmul(ps, xh[k][:, mt*128:(mt+1)*128], W['wv'][k],
                                     start=(k == 0), stop=(k == KT - 1))
                kti = c * (CHUNK // 128) + mt
                vt = apool.tile([128, HL * VS], dt.bfloat16,
                                name=f"v{kti}", tag=f"v{kti}", bufs=1)
                vv = vt.rearrange("p (h e) -> p h e", e=VS)
                nc.scalar.activation(out=vv[:, :, 0:D],
                                     in_=ps.rearrange("p (h e) -> p h e", e=D),
                                     func=AF.Copy)
                nc.vector.memset(vv[:, :, D:D+1], 1.0)
                if bias_on["bv"]:
                    nc.vector.tensor_add(
                        out=vv[:, :, 0:D], in0=vv[:, :, 0:D],
                        in1=bias_t["bv"].rearrange("p (h e) -> p h e", e=D))
                v_store[kti] = vt
                yield

        def qkv_block(W, c, xh, kt_store, v_store):
            qt2 = []
            for _ in qkv_gen(W, c, xh, kt_store, v_store, qt2):
                pass
            return qt2

        def attn_block(c, qt2, kt_store, v_store, ot_sb=None, heads=None,
                       filler=None, fill_per_head=2):
            """Per head: S^T -> exp -> mask -> O^T (ones column gives denom).

            `filler`: optional generator whose emission is interleaved between
            heads (independent PE work to fill exp-wait bubbles)."""
            if ot_sb is None:
                ot_sb = [apool.tile([128, CHUNK], dt.bfloat16, name=f"oT{j}",
                                    tag=f"oT{j}") for j in range(CL // 128)]
            nkt = (c + 1) * (CHUNK // 128)
            for h in (heads if heads is not None else range(HL)):
                if filler is not None:
                    for _ in range(fill_per_head):
                        next(filler, None)
                j, half = h // 2, h % 2
                pr = slice(half * 64, half * 64 + 64)
                ot_ps = ps_ot.tile([65, CHUNK], dt.float32, tag="ot")
                for kt in range(nkt):
                    kc, km = kt // (CHUNK // 128), kt % (CHUNK // 128)
                    s_ps = ps_mm.tile([128, CHUNK], dt.float32, tag="mm")
                    nc.tensor.matmul(s_ps,
                                     kt_store[(kc, j)][pr, km*128:(km+1)*128],
                                     qt2[j][pr, :], start=True, stop=True)
                    es = espool.tile([128, CHUNK], dt.bfloat16, name="es", tag="es")
                    nc.scalar.activation(out=es, in_=s_ps, func=AF.Exp)
                    rel = kt * 128 - c * CHUNK
                    if rel >= 0:
                        nc.vector.tensor_mul(out=es, in0=es, in1=mask_t[rel // 128])
                    nc.tensor.matmul(ot_ps,
                                     v_store[kt][:, h*VS:h*VS+D+1], es,
                                     start=(kt == 0), stop=(kt == nkt - 1))
                recip = spool.tile([1, CHUNK], dt.float32, tag="recip")
                nc.vector.reciprocal(out=recip, in_=ot_ps[64:65, :])
                if pool_bcast:
                    rb_sb = spool.tile([64, CHUNK], dt.float32, tag="rb_sb")
                    nc.gpsimd.partition_broadcast(rb_sb, recip)
                else:
                    recb = spool.tile([1, CHUNK], dt.bfloat16, tag="recb")
                    nc.scalar.activation(out=recb, in_=recip, func=AF.Copy)
                    rb_ps = ps_bc.tile([128, CHUNK], dt.float32, tag="bc")
                    nc.tensor.matmul(rb_ps[0:64, :], ones_row[:, 0:64], recb,
                                     start=True, stop=True)
                    rb_sb = spool.tile([64, CHUNK], dt.bfloat16, tag="rb_sb")
                    nc.scalar.activation(out=rb_sb, in_=rb_ps[0:64, :], func=AF.Copy)
                nc.vector.tensor_mul(out=ot_sb[j][half*64:(half+1)*64, :],
                                     in0=ot_ps[0:64, :], in1=rb_sb)
            return ot_sb

        def proj_block(W, c, ot_sb):
            bias_t = W['bias']
            stw = stpool.tile([128, KT * CHUNK], dt.bfloat16, name="prst",
                              tag="stage", bufs=2)
            for mt in range(KT):
                ps = ps_mm.tile([128, CHUNK], dt.float32, tag="mm")
                for j in range(CL // 128):
                    nc.tensor.matmul(ps, W['wpr'][j][:, mt*128:(mt+1)*128], ot_sb[j],
                                     start=(j == 0), stop=(j == CL // 128 - 1))
                st = stw[:, mt*CHUNK:(mt+1)*CHUNK]
                if bias_on["bpr"]:
                    nc.scalar.activation(out=st, in_=ps, func=AF.Identity,
                                         bias=bias_t["bpr"][:, mt:mt+1])
                else:
                    nc.scalar.activation(out=st, in_=ps, func=AF.Copy)
            return stw

        def ffn_block(W, c, xh2):
            bias_t = W['bias']
            a_sb = []
            for mt in range(HIDL // 128):
                ps = ps_mm.tile([128, CHUNK], dt.float32, tag="mm")
                for k in range(KT):
                    nc.tensor.matmul(ps, W['wf1'][k][:, mt*128:(mt+1)*128], xh2[k],
                                     start=(k == 0), stop=(k == KT - 1))
                at = hpool.tile([128, CHUNK], dt.bfloat16, name=f"ga{mt}",
                                tag=f"ga{mt}", bufs=1)
                if gelu_mode == "exact":
                    if bias_on["bf1"]:
                        nc.scalar.activation(out=at, in_=ps, func=AF.Gelu,
                                             bias=bias_t["bf1"][:, mt:mt+1])
                    else:
                        nc.scalar.activation(out=at, in_=ps, func=AF.Gelu)
                else:
                    assert not bias_on["bf1"]
                    sg = sqpool.tile([128, CHUNK], dt.bfloat16, name="sg", tag="sq")
                    nc.scalar.activation(out=sg, in_=ps, func=AF.Sigmoid,
                                         scale=1.702)
                    nc.vector.tensor_mul(out=at, in0=sg, in1=ps)
                a_sb.append(at)
            stw = stpool.tile([128, KT * CHUNK], dt.bfloat16, name="f2st",
                              tag="stage", bufs=2)
            for mt in range(KT):
                ps = ps_mm.tile([128, CHUNK], dt.float32, tag="mm")
                for k in range(HIDL // 128):
                    nc.tensor.matmul(ps, W['wf2'][k][:, mt*128:(mt+1)*128], a_sb[k],
                                     start=(k == 0), stop=(k == HIDL // 128 - 1))
                st = stw[:, mt*CHUNK:(mt+1)*CHUNK]
                if bias_on["bf2"]:
                    nc.scalar.activation(out=st, in_=ps, func=AF.Identity,
                                         bias=bias_t["bf2"][:, mt:mt+1])
                else:
                    nc.scalar.activation(out=st, in_=ps, func=AF.Copy)
            return stw

        # Two-chunk software pipeline: each AllReduce is issued before ~80us of
        # independent compute from the other chunk, so its latency (plus the
        # serial residual->LN chain behind it) hides completely.
        pending = [None] * NCHUNK      # previous layer's fc2 AllReduces
        for li in range(L * loop_mult):
            l = li % L
            W = load_weights(l)
            kt_store, v_store = {}, {}
            ar_pr = [None] * NCHUNK
            up0 = up1 = None
            if pending[0] is not None:
                up0 = load_upd(pending[0], "f2p0")
            xh0 = layer_norm(f"ln1_{l}_0", 0, up0)
            qt2_0 = qkv_block(W, 0, xh0, kt_store, v_store)
            if interleave:
                ot0 = attn_block(0, qt2_0, kt_store, v_store, heads=[0, 1])
                if pending[1] is not None:
                    up1 = load_upd(pending[1], "f2p1")
                xh1 = layer_norm(f"ln1_{l}_1", 1, up1)
                qt2_1 = []
                filler = qkv_gen(W, 1, xh1, kt_store, v_store, qt2_1)
                attn_block(0, qt2_0, kt_store, v_store, ot_sb=ot0, heads=[2, 3],
                           filler=filler, fill_per_head=4)
                for _ in filler:
                    pass
                ar_pr[0] = all_reduce(proj_block(W, 0, ot0), f"pr_{l}_0")
            else:
                ot0 = attn_block(0, qt2_0, kt_store, v_store)
                ar_pr[0] = all_reduce(proj_block(W, 0, ot0), f"pr_{l}_0")
                if pending[1] is not None:
                    up1 = load_upd(pending[1], "f2p1")
                xh1 = layer_norm(f"ln1_{l}_1", 1, up1)
                qt2_1 = qkv_block(W, 1, xh1, kt_store, v_store)
            pending = [None] * NCHUNK
            ot1 = attn_block(1, qt2_1, kt_store, v_store)
            ar_pr[1] = all_reduce(proj_block(W, 1, ot1), f"pr_{l}_1")
            for c in range(NCHUNK):
                up = load_upd(ar_pr[c], f"pr{c}")
                xh2 = layer_norm(f"ln2_{l}_{c}", c, up, nxt=AF.Gelu)
                stage2 = ffn_block(W, c, xh2)
                pending[c] = all_reduce(stage2, f"f2_{l}_{c}")
        # Epilogue: fold the last AllReduce into the fp32 output copy directly
        # (one less bf16 rounding on the final residual add).
        fin_up = [load_upd(pending[c], f"fin{c}") for c in range(NCHUNK)]
        for k in range(KT):
            of = stpool.tile([128, N], dt.float32, name=f"of{k}", tag="ofin", bufs=2)
            for c in range(NCHUNK):
                ts = slice(c * CHUNK, (c + 1) * CHUNK)
                nc.vector.tensor_add(out=of[:, ts], in0=xt[k][:, ts],
                                     in1=fin_up[c][:, k*CHUNK:(k+1)*CHUNK])
            nc.sync.dma_start(out=out_p[k], in_=of)

    if not nc.is_finalized():
        nc.finalize()
    return nc


def _prep_core_inputs(inputs, folded):
    """Per-core in_maps (host-side sharding + layout + bf16 cast)."""
    x = np.asarray(inputs['x'], np.float32)
    masks = _make_masks()
    scale = np.float32(D ** -0.5)

    per_core = []
    bias_on = {k: False for k in ("bqk", "bv", "bpr", "bf1", "bf2")}
    shard_cache = {}
    for cid in range(NCORES):
        r, b = cid % TP, cid // TP
        if r not in shard_cache:
            wqk, wv, wpr, wf1, wf2 = [], [], [], [], []
            bqk, bv, bpr, bf1, bf2 = [], [], [], [], []
            for l in range(L):
                F = folded[l]
                Wq = F['Wqkv_t'][:, r*CL:(r+1)*CL] * scale
                Wk = F['Wqkv_t'][:, C + r*CL: C + (r+1)*CL]
                Wv = F['Wqkv_t'][:, 2*C + r*CL: 2*C + (r+1)*CL]
                wqk.append(np.concatenate([Wq, Wk], axis=1).reshape(KT, 128, 2*CL))
                wv.append(Wv.reshape(KT, 128, CL))
                wpr.append(F['Wproj_t'][r*CL:(r+1)*CL, :].reshape(CL//128, 128, C))
                wf1.append(F['Wfc1_t'][:, r*HIDL:(r+1)*HIDL].reshape(KT, 128, HIDL))
                wf2.append(F['Wfc2_t'][r*HIDL:(r+1)*HIDL, :].reshape(HIDL//128, 128, C))
                bq = F['bqkv'][r*CL:(r+1)*CL] * scale
                bk = F['bqkv'][C + r*CL: C + (r+1)*CL]
                bqk.append(np.concatenate([bq, bk]).reshape(4, 128).T)
                bv.append(np.broadcast_to(
                    F['bqkv'][2*C + r*CL: 2*C + (r+1)*CL], (128, CL)).copy())
                bpr.append(F['bproj'].reshape(KT, 128).T / TP)
                bf1.append(F['bfc1'][r*HIDL:(r+1)*HIDL].reshape(KT, 128).T)
                bf2.append(F['bfc2'].reshape(KT, 128).T / TP)
            shard = dict(
                wqk=np.stack(wqk).astype(BF16), wv=np.stack(wv).astype(BF16),
                wpr=np.stack(wpr).astype(BF16), wf1=np.stack(wf1).astype(BF16),
                wf2=np.stack(wf2).astype(BF16),
                bqk=np.stack(bqk).astype(np.float32), bv=np.stack(bv).astype(np.float32),
                bpr=np.stack(bpr).astype(np.float32), bf1=np.stack(bf1).astype(np.float32),
                bf2=np.stack(bf2).astype(np.float32))
            shard_cache[r] = shard
        shard = shard_cache[r]
        m = dict(shard)
        m['xT'] = np.ascontiguousarray(x[b].T).reshape(KT, 128, N).astype(BF16)
        m['masks'] = masks
        per_core.append(m)

    for nm in bias_on:
        bias_on[nm] = any(bool(np.abs(m[nm]).max() > 0) for m in per_core)
    for m in per_core:
        for nm in list(m):
            if nm in bias_on and not bias_on[nm]:
                del m[nm]
    return per_core, bias_on


LAST_RESULT = None


def kernel(**inputs):
    global LAST_RESULT
    from concourse.bass_utils import run_bass_kernel_spmd
    folded = _fold_weights(inputs)
    in_maps, bias_on = _prep_core_inputs(inputs, folded)
    nc = build_program(bias_on)
    res = run_bass_kernel_spmd(nc, in_maps, core_ids=list(range(NCORES)))
    LAST_RESULT = res
    outs = []
    for b in range(B):
        o = res.results[b * TP]["out"].reshape(C, N).T    # [tokens, C]
        outs.append(o)
    return np.stack(outs).astype(np.float32)


if __name__ == "__main__":
    import reference
    inp = reference.setup_inputs()
    out = kernel(**{k: np.asarray(v) for k, v in inp.items()})
    exp = np.asarray(reference.reference(**inp))
    err = np.abs(out - exp).max() / np.abs(exp).max()
    print("Relative error:", err)



# revision 61
# speedup vs baseline: 6.2027x; 1.0412x over previous
"""Trainium2 Bass kernel for nn_CP_TransformerDecoder_Action.

Strategy
--------
Host side (numpy, not timed):
  * The CP adapters and LN affine params are *linear*, so they fold exactly into
    the dense per-layer weights:  Wqkv_eff, Wproj_eff, Wfc1_eff, Wfc2_eff.
  * DP=2 (batch) x TP=4 (heads / hidden) sharding across 8 cores.
  * Weights pre-transposed to matmul lhsT layout, cast to bf16, pre-tiled.
  * Residual stream is kept FEATURE-major (xT [C, tokens]) on device so every
    matmul contracts over the partition dim with zero on-device transposes.

Device (one SPMD program, 8 cores):
  per layer:  LN1 -> qkT/kT (transposed) + v (token-major) -> S^T = k q^T ->
  exp -> mask -> O^T = v_aug^T P^T (ones-column gives softmax denom) ->
  normalize -> proj partial -> bf16 AllReduce(4-core group) -> residual ->
  LN2 -> fc1+gelu -> fc2 partial -> AllReduce -> residual.

  Tokens are processed in 2 chunks of 512 arranged as a software pipeline:
  each AllReduce is issued before ~80us of independent compute from the other
  chunk, hiding collective latency and the serial residual->LN chains.
  AllReduce staging uses one wide [128,4096] tile and a single DMA per
  direction (staging on the gpsimd queue, result fetch on SP, so weight
  prefetch never queues behind a collective). The residual stream is bf16
  (tolerance is 2e-2); the residual add is fused into the LN's bf16 working
  copy, with the bf16 commit emitted off the critical path. rstd uses
  exp(-0.5*ln(var+eps)) with a dummy [1,1] Ln prefetching the ACT table while
  the AllReduce is in flight.
"""

import numpy as np
import ml_dtypes

L, B, N, C, H, D, R = 4, 2, 1024, 1024, 16, 64, 64
HID = 4 * C
TP = 4                      # tensor-parallel group size
NCORES = 8
CHUNK = 512                 # token chunk (matmul moving free dim)
NCHUNK = N // CHUNK         # 2
KT = C // 128               # 8 C-tiles
HL = H // TP                # 4 heads per core
CL = HL * D                 # 256 local attention features
HIDL = HID // TP            # 1024 local hidden
RG = [[0, 1, 2, 3], [4, 5, 6, 7]]
VS = D + 4                  # v storage stride per head (64 data + 1 ones + pad)

BF16 = ml_dtypes.bfloat16


def _fold_weights(inp):
    """Fold LN affine + CP adapters into dense per-layer weights (fp32 exact)."""
    f32 = np.float32
    u_w = np.asarray(inp['u_w'], f32)       # [R, C]
    v_w = np.asarray(inp['v_w'], f32)       # [C, R]
    cp_c = np.asarray(inp['cp_c'], f32)     # [R, R, R]
    out = []
    for l in range(L):
        g1 = np.asarray(inp['ln1_g'][l], f32); b1 = np.asarray(inp['ln1_b'][l], f32)
        g2 = np.asarray(inp['ln2_g'][l], f32); b2 = np.asarray(inp['ln2_b'][l], f32)
        qkv_w = np.asarray(inp['qkv_w'][l], f32)
        proj_w = np.asarray(inp['proj_w'][l], f32)
        fc1_w = np.asarray(inp['fc1_w'][l], f32)
        fc2_w = np.asarray(inp['fc2_w'][l], f32)
        CPa = np.einsum('abr,rf->abf', cp_c, np.asarray(inp['cp_att'][l], f32))
        CPm = np.einsum('abr,rf->abf', cp_c, np.asarray(inp['mlp_cp'][l], f32))

        Pcat = np.concatenate([CPa[:, :, i] @ v_w.T for i in range(3)], axis=1)   # [R,3C]
        Wqkv_t = (qkv_w * g1[None, :]).T + (u_w * g1[None, :]).T @ Pcat           # [C,3C]
        bqkv = b1 @ qkv_w.T + (b1 @ u_w.T) @ Pcat                                  # [3C]

        Wproj_t = proj_w.T + u_w.T @ (CPa[:, :, 3] @ v_w.T)                        # [C,C]
        bproj = np.asarray(inp['proj_b'][l], f32)

        fc1_cp = CPm[:, :, :4].reshape(R, 4 * R)
        T = np.concatenate([fc1_cp[:, j*R:(j+1)*R] @ v_w.T for j in range(4)], axis=1)
        Wfc1_t = (fc1_w * g2[None, :]).T + (u_w * g2[None, :]).T @ T               # [C,HID]
        bfc1 = np.asarray(inp['fc1_b'][l], f32) + b2 @ fc1_w.T + (b2 @ u_w.T) @ T

        fc2_cp = CPm[:, :, 4:].reshape(R, 4 * R)
        Z = np.concatenate([u_w.T @ fc2_cp[:, j*R:(j+1)*R].T @ v_w.T for j in range(4)], axis=0)
        Wfc2_t = fc2_w.T + Z                                                       # [HID,C]
        bfc2 = np.asarray(inp['fc2_b'][l], f32)
        out.append(dict(Wqkv_t=Wqkv_t, bqkv=bqkv, Wproj_t=Wproj_t, bproj=bproj,
                        Wfc1_t=Wfc1_t, bfc1=bfc1, Wfc2_t=Wfc2_t, bfc2=bfc2))
    return out


def _make_masks():
    """Causal multipliers for diagonal-crossing S^T tiles, rel = key0 - query0."""
    kk = np.arange(128)[:, None]
    qq = np.arange(CHUNK)[None, :]
    return np.stack([(p * 128 + kk) <= qq for p in range(CHUNK // 128)]).astype(BF16)


def build_program(bias_on, gelu_mode="exact", collective_mode="on", loop_mult=1,
                  pool_commit=False, pool_bcast=False, interleave=False):
    """Build the SPMD Bass/Tile program. bias_on: dict of bools per bias kind.

    gelu_mode="approx" replaces the ACT Gelu table with x*sigmoid(1.702x) so
    the kernel can run under CoreSim (which lacks Gelu); hardware uses "exact".
    """
    from contextlib import ExitStack
    import concourse.mybir as mybir
    import concourse.tile as tile
    from concourse import bacc

    dt = mybir.dt
    AF = mybir.ActivationFunctionType
    nc = bacc.Bacc(num_devices=NCORES)

    xT_p = nc.declare_dram_parameter("xT", [KT, 128, N], dt.bfloat16, isOutput=False)
    wqk_p = nc.declare_dram_parameter("wqk", [L, KT, 128, 2 * CL], dt.bfloat16, isOutput=False)
    wv_p = nc.declare_dram_parameter("wv", [L, KT, 128, CL], dt.bfloat16, isOutput=False)
    wpr_p = nc.declare_dram_parameter("wpr", [L, CL // 128, 128, C], dt.bfloat16, isOutput=False)
    wf1_p = nc.declare_dram_parameter("wf1", [L, KT, 128, HIDL], dt.bfloat16, isOutput=False)
    wf2_p = nc.declare_dram_parameter("wf2", [L, HIDL // 128, 128, C], dt.bfloat16, isOutput=False)
    w1f1_p = nc.declare_dram_parameter("w1f1", [L, 1, HIDL], dt.bfloat16, isOutput=False)
    mask_p = nc.declare_dram_parameter("masks", [4, 128, CHUNK], dt.bfloat16, isOutput=False)
    bias_p = {}
    for nm, shp in (("bqk", [L, 128, 4]), ("bv", [L, 128, CL]),
                    ("bpr", [L, 128, KT]), ("bf1", [L, 128, KT]), ("bf2", [L, 128, KT])):
        if bias_on[nm]:
            bias_p[nm] = nc.declare_dram_parameter(nm, shp, dt.float32, isOutput=False)
    out_p = nc.declare_dram_parameter("out", [KT, 128, N], dt.float32, isOutput=True)

    with tile.TileContext(nc) as tc, ExitStack() as ctx:
        consts = ctx.enter_context(tc.tile_pool(name="consts", bufs=1))
        wpool = ctx.enter_context(tc.tile_pool(name="wpool", bufs=2))
        wfpool = ctx.enter_context(tc.tile_pool(name="wfpool", bufs=1))
        xpool = ctx.enter_context(tc.tile_pool(name="xpool", bufs=1))
        hpool = ctx.enter_context(tc.tile_pool(name="hpool", bufs=2))    # xb/xh, a
        apool = ctx.enter_context(tc.tile_pool(name="apool", bufs=2))    # attn tiles
        espool = ctx.enter_context(tc.tile_pool(name="espool", bufs=3))
        stpool = ctx.enter_context(tc.tile_pool(name="stpool", bufs=4))  # staging
        spool = ctx.enter_context(tc.tile_pool(name="spool", bufs=2))    # small stats
        sqpool = ctx.enter_context(tc.tile_pool(name="sqpool", bufs=3))
        ps_mm = ctx.enter_context(tc.tile_pool(name="ps_mm", bufs=3, space="PSUM"))
        ps_ot = ctx.enter_context(tc.tile_pool(name="ps_ot", bufs=2, space="PSUM"))
        ps_bc = ctx.enter_context(tc.tile_pool(name="ps_bc", bufs=2, space="PSUM"))
        ps_st = ctx.enter_context(tc.tile_pool(name="ps_st", bufs=1, space="PSUM"))
        dram = ctx.enter_context(tc.tile_pool(name="dram", bufs=4, space="DRAM"))

        # ---- constants
        ones_col = consts.tile([128, 1], dt.bfloat16)
        nc.vector.memset(ones_col, 1.0)
        mones_col = consts.tile([128, 1], dt.bfloat16)
        nc.vector.memset(mones_col, -1.0)
        ones_row = consts.tile([1, 128], dt.bfloat16)
        nc.vector.memset(ones_row, 1.0)
        mones_row = consts.tile([1, 128], dt.bfloat16)
        nc.vector.memset(mones_row, -1.0)
        eps_t = consts.tile([1, 1], dt.float32)
        nc.vector.memset(eps_t, 1e-5)
        mask_t = []
        for p in range(4):
            mt_ = consts.tile([128, CHUNK], dt.bfloat16, name=f"mask{p}")
            nc.sync.dma_start(out=mt_, in_=mask_p[p])
            mask_t.append(mt_)

        # ---- residual stream, feature-major bf16 (fp32 not needed at 2e-2 tol)
        xt = []
        for k in range(KT):
            t = xpool.tile([128, N], dt.bfloat16, name=f"x{k}")
            nc.sync.dma_start(out=t, in_=xT_p[k])
            xt.append(t)

        def layer_norm(lname, c, up=None, nxt=AF.Exp, raw=False):
            """Returns list of 8 bf16 tiles xh[k] = normalized x chunk, [128, CHUNK].

            If `up` (a wide [128, KT*CHUNK] bf16 AllReduce result) is given, the
            residual add is fused into the bf16 working copy (critical path) and
            the commit into xt is deferred off the critical path.
            Dummy [1,1] activations prefetch the sqrt/exp ACT tables while the
            engine is otherwise idle, so no table load sits on the serial
            stats->rstd->apply chain."""
            ts = slice(c * CHUNK, (c + 1) * CHUNK)
            dum = spool.tile([1, 1], dt.float32, tag="dum")
            nc.scalar.activation(out=dum, in_=eps_t, func=AF.Ln)
            xb = []
            for k in range(KT):
                t = hpool.tile([128, CHUNK], dt.bfloat16, name=f"xb{k}", tag=f"xb{k}")
                if up is None:
                    nc.vector.tensor_copy(out=t, in_=xt[k][:, ts])
                else:
                    nc.vector.tensor_add(out=t, in0=xt[k][:, ts],
                                         in1=up[:, k*CHUNK:(k+1)*CHUNK])
                xb.append(t)
            stat = ps_st.tile([64, CHUNK], dt.float32, tag="stat")
            for k in range(KT):
                nc.tensor.matmul(stat[0:1, :], mones_col, xb[k],
                                 start=(k == 0), stop=(k == KT - 1))
            for k in range(KT):
                sq = sqpool.tile([128, CHUNK], dt.bfloat16, name="sq", tag="sq")
                nc.vector.tensor_mul(out=sq, in0=xb[k], in1=xb[k])
                nc.tensor.matmul(stat[32:33, :], ones_col, sq,
                                 start=(k == 0), stop=(k == KT - 1))
            if up is not None:   # deferred fp32 residual commit, off critical path
                eng = nc.gpsimd if pool_commit else nc.vector
                for k in range(KT):
                    eng.tensor_add(out=xt[k][:, ts], in0=xt[k][:, ts],
                                   in1=up[:, k*CHUNK:(k+1)*CHUNK])
            sm = spool.tile([1, CHUNK], dt.float32, tag="sm")    # -mean
            nc.scalar.activation(out=sm, in_=stat[0:1, :], func=AF.Copy,
                                 scale=1.0 / C)
            msq = spool.tile([1, CHUNK], dt.float32, tag="msq")  # meansq
            nc.scalar.activation(out=msq, in_=stat[32:33, :], func=AF.Copy,
                                 scale=1.0 / C)
            m2 = spool.tile([1, CHUNK], dt.float32, tag="m2")
            nc.vector.tensor_mul(out=m2, in0=sm, in1=sm)
            var = spool.tile([1, CHUNK], dt.float32, tag="var")
            nc.vector.tensor_sub(out=var, in0=msq, in1=m2)
            lnv = spool.tile([1, CHUNK], dt.float32, tag="lnv")
            nc.scalar.activation(out=lnv, in_=var, func=AF.Ln, bias=eps_t[:, 0:1])
            a_bf = spool.tile([1, CHUNK], dt.bfloat16, tag="a_bf")   # rstd
            nc.scalar.activation(out=a_bf, in_=lnv, func=AF.Exp, scale=-0.5)
            b_bf = spool.tile([1, CHUNK], dt.bfloat16, tag="b_bf")   # -mean*rstd
            nc.vector.tensor_mul(out=b_bf, in0=sm, in1=a_bf)
            ps_a = ps_bc.tile([128, CHUNK], dt.float32, tag="bc")
            nc.tensor.matmul(ps_a, ones_row, a_bf, start=True, stop=True)
            a_bc = spool.tile([128, CHUNK], dt.bfloat16, tag="a_bc")
            nc.scalar.activation(out=a_bc, in_=ps_a, func=AF.Copy)
            if raw:
                # Deferred normalization: caller folds -mean via a rank-1
                # matmul (w1 (x) b_bf) and the rstd scale via a_bc at PSUM
                # readout, so its matmuls can start right after the adds.
                return xb, a_bc, b_bf
            ps_b = ps_bc.tile([128, CHUNK], dt.float32, tag="bc")
            nc.tensor.matmul(ps_b, ones_row, b_bf, start=True, stop=True)
            b_bc = spool.tile([128, CHUNK], dt.bfloat16, tag="b_bc")
            nc.scalar.activation(out=b_bc, in_=ps_b, func=AF.Copy)
            for k in range(KT):
                nc.vector.tensor_mul(out=xb[k], in0=xb[k], in1=a_bc)
                nc.vector.tensor_add(out=xb[k], in0=xb[k], in1=b_bc)
            return xb

        def all_reduce(stage_wide, lname):
            """AllReduce one wide [128, KT*CHUNK] bf16 staging tile.

            One staging DMA on the gpsimd (Pool) queue so neither the SP queue
            (weight prefetch) nor compute queues block behind the collective."""
            arin = dram.tile([128, KT * CHUNK], dt.bfloat16, name="arin", tag="arin")
            arout = dram.tile([128, KT * CHUNK], dt.bfloat16, name="arout", tag="arout")
            nc.gpsimd.dma_start(out=arin, in_=stage_wide)
            if collective_mode == "on":
                nc.gpsimd.collective_compute(
                    "AllReduce", mybir.AluOpType.add, replica_groups=RG,
                    ins=[arin.opt()], outs=[arout.opt()])
            else:  # timing ablation: local copy instead of AllReduce (wrong math)
                nc.gpsimd.dma_start(out=arout.opt(), in_=arin.opt())
            return arout

        def load_upd(arout, tag):
            """Fetch an AllReduce result with one wide DMA on the SP queue."""
            up = stpool.tile([128, KT * CHUNK], dt.bfloat16, name=f"upd{tag}",
                             tag="upd", bufs=2)
            nc.sync.dma_start(out=up, in_=arout)
            return up

        def resid_update(arout, c, tag):
            """Epilogue-only: load AllReduce result and add into the residual."""
            ts = slice(c * CHUNK, (c + 1) * CHUNK)
            up = load_upd(arout, tag)
            for mt in range(KT):
                nc.vector.tensor_add(out=xt[mt][:, ts], in0=xt[mt][:, ts],
                                     in1=up[:, mt*CHUNK:(mt+1)*CHUNK])

        def load_weights(l):
            """Weight DMAs on the SP queue (pure prefetch, no collective deps)."""
            W = {}
            wqk = []
            for k in range(KT):
                t = wpool.tile([128, 2 * CL], dt.bfloat16, name=f"wqk{k}", tag=f"wqk{k}")
                nc.sync.dma_start(out=t, in_=wqk_p[l, k])
                wqk.append(t)
            wv = []
            for k in range(KT):
                t = wpool.tile([128, CL], dt.bfloat16, name=f"wv{k}", tag=f"wv{k}", bufs=1)
                nc.sync.dma_start(out=t, in_=wv_p[l, k])
                wv.append(t)
            wpr = []
            for j in range(CL // 128):
                t = wpool.tile([128, C], dt.bfloat16, name=f"wpr{j}", tag=f"wpr{j}", bufs=1)
                nc.sync.dma_start(out=t, in_=wpr_p[l, j])
                wpr.append(t)
            wf1 = []
            for k in range(KT):
                t = wfpool.tile([128, HIDL], dt.bfloat16, name=f"wf1{k}", tag=f"wf1{k}")
                nc.sync.dma_start(out=t, in_=wf1_p[l, k])
                wf1.append(t)
            wf2 = []
            for k in range(HIDL // 128):
                t = wfpool.tile([128, C], dt.bfloat16, name=f"wf2{k}", tag=f"wf2{k}")
                nc.sync.dma_start(out=t, in_=wf2_p[l, k])
                wf2.append(t)
            w1t = wpool.tile([1, HIDL], dt.bfloat16, name="w1f1", tag="w1f1")
            nc.sync.dma_start(out=w1t, in_=w1f1_p[l])
            bias_t = {}
            for nm in bias_p:
                t = wpool.tile(list(bias_p[nm].shape[1:]), dt.float32,
                               name=f"{nm}t", tag=f"{nm}t")
                nc.sync.dma_start(out=t, in_=bias_p[nm][l])
                bias_t[nm] = t
            W.update(wqk=wqk, wv=wv, wpr=wpr, wf1=wf1, wf2=wf2, w1f1=w1t,
                     bias=bias_t)
            return W

        def qkv_gen(W, c, xh, kt_store, v_store, qt2_out):
            """Generator: qT/kT (2 heads per [128,CHUNK] tile) + token-major v.

            Yields after each matmul group so the caller can interleave the
            emission with another chunk's attention (fills PE while ACT does
            that chunk's exps)."""
            bias_t = W['bias']
            for mt in range(2 * CL // 128):   # 4 Mtiles: q q k k
                ps = ps_mm.tile([128, CHUNK], dt.float32, tag="mm")
                for k in range(KT):
                    nc.tensor.matmul(ps, W['wqk'][k][:, mt*128:(mt+1)*128], xh[k],
                                     start=(k == 0), stop=(k == KT - 1))
                j = mt % 2
                if mt < 2:
                    dst = apool.tile([128, CHUNK], dt.bfloat16,
                                     name=f"qT{j}", tag=f"qT{j}")
                    qt2_out.append(dst)
                else:
                    dst = apool.tile([128, CHUNK], dt.bfloat16,
                                     name=f"kT{c}_{j}", tag=f"kT{c}_{j}", bufs=1)
                    kt_store[(c, j)] = dst
                if bias_on["bqk"]:
                    nc.scalar.activation(out=dst, in_=ps, func=AF.Identity,
                                         bias=bias_t["bqk"][:, mt:mt+1])
                else:
                    nc.scalar.activation(out=dst, in_=ps, func=AF.Copy)
                yield

            for mt in range(CHUNK // 128):
                ps = ps_mm.tile([128, CL], dt.float32, tag="mm")
                for k in range(KT):
                    nc.tensor.matmul(ps, xh[k][:, mt*128:(mt+1)*128], W['wv'][k],
                                     start=(k == 0), stop=(k == KT - 1))
                kti = c * (CHUNK // 128) + mt
                vt = apool.tile([128, HL * VS], dt.bfloat16,
                                name=f"v{kti}", tag=f"v{kti}", bufs=1)
                vv = vt.rearrange("p (h e) -> p h e", e=VS)
                nc.scalar.activation(out=vv[:, :, 0:D],
                                     in_=ps.rearrange("p (h e) -> p h e", e=D),
                                     func=AF.Copy)
                nc.vector.memset(vv[:, :, D:D+1], 1.0)
                if bias_on["bv"]:
                    nc.vector.tensor_add(
                        out=vv[:, :, 0:D], in0=vv[:, :, 0:D],
                        in1=bias_t["bv"].rearrange("p (h e) -> p h e", e=D))
                v_store[kti] = vt
                yield

        def qkv_block(W, c, xh, kt_store, v_store):
            qt2 = []
            for _ in qkv_gen(W, c, xh, kt_store, v_store, qt2):
                pass
            return qt2

        def attn_block(c, qt2, kt_store, v_store, ot_sb=None, heads=None,
                       filler=None, fill_per_head=2):
            """Per head: S^T -> exp -> mask -> O^T (ones column gives denom).

            `filler`: optional generator whose emission is interleaved between
            heads (independent PE work to fill exp-wait bubbles)."""
            if ot_sb is None:
                ot_sb = [apool.tile([128, CHUNK], dt.bfloat16, name=f"oT{j}",
                                    tag=f"oT{j}") for j in range(CL // 128)]
            nkt = (c + 1) * (CHUNK // 128)
            for h in (heads if heads is not None else range(HL)):
                if filler is not None:
                    for _ in range(fill_per_head):
                        next(filler, None)
                j, half = h // 2, h % 2
                pr = slice(half * 64, half * 64 + 64)
                ot_ps = ps_ot.tile([65, CHUNK], dt.float32, tag="ot")
                for kt in range(nkt):
                    kc, km = kt // (CHUNK // 128), kt % (CHUNK // 128)
                    s_ps = ps_mm.tile([128, CHUNK], dt.float32, tag="mm")
                    nc.tensor.matmul(s_ps,
                                     kt_store[(kc, j)][pr, km*128:(km+1)*128],
                                     qt2[j][pr, :], start=True, stop=True)
                    es = espool.tile([128, CHUNK], dt.bfloat16, name="es", tag="es")
                    nc.scalar.activation(out=es, in_=s_ps, func=AF.Exp)
                    rel = kt * 128 - c * CHUNK
                    if rel >= 0:
                        nc.vector.tensor_mul(out=es, in0=es, in1=mask_t[rel // 128])
                    nc.tensor.matmul(ot_ps,
                                     v_store[kt][:, h*VS:h*VS+D+1], es,
                                     start=(kt == 0), stop=(kt == nkt - 1))
                recip = spool.tile([1, CHUNK], dt.float32, tag="recip")
                nc.vector.reciprocal(out=recip, in_=ot_ps[64:65, :])
                if pool_bcast:
                    rb_sb = spool.tile([64, CHUNK], dt.float32, tag="rb_sb")
                    nc.gpsimd.partition_broadcast(rb_sb, recip)
                else:
                    recb = spool.tile([1, CHUNK], dt.bfloat16, tag="recb")
                    nc.scalar.activation(out=recb, in_=recip, func=AF.Copy)
                    rb_ps = ps_bc.tile([128, CHUNK], dt.float32, tag="bc")
                    nc.tensor.matmul(rb_ps[0:64, :], ones_row[:, 0:64], recb,
                                     start=True, stop=True)
                    rb_sb = spool.tile([64, CHUNK], dt.bfloat16, tag="rb_sb")
                    nc.scalar.activation(out=rb_sb, in_=rb_ps[0:64, :], func=AF.Copy)
                nc.vector.tensor_mul(out=ot_sb[j][half*64:(half+1)*64, :],
                                     in0=ot_ps[0:64, :], in1=rb_sb)
            return ot_sb

        def proj_block(W, c, ot_sb):
            bias_t = W['bias']
            stw = stpool.tile([128, KT * CHUNK], dt.bfloat16, name="prst",
                              tag="stage", bufs=2)
            for mt in range(KT):
                ps = ps_mm.tile([128, CHUNK], dt.float32, tag="mm")
                for j in range(CL // 128):
                    nc.tensor.matmul(ps, W['wpr'][j][:, mt*128:(mt+1)*128], ot_sb[j],
                                     start=(j == 0), stop=(j == CL // 128 - 1))
                st = stw[:, mt*CHUNK:(mt+1)*CHUNK]
                if bias_on["bpr"]:
                    nc.scalar.activation(out=st, in_=ps, func=AF.Identity,
                                         bias=bias_t["bpr"][:, mt:mt+1])
                else:
                    nc.scalar.activation(out=st, in_=ps, func=AF.Copy)
            return stw

        def ffn_block(W, c, xh2, a_bc=None, b_bf=None):
            bias_t = W['bias']
            a_sb = []
            if a_bc is not None:
                # Deferred LN: fc1 accumulates over raw xb (starts right after
                # the residual adds, parallel with the LN stats); the -mean
                # term lands as a rank-1 (w1 (x) b_bf) accumulation and rstd
                # as a per-token multiply at PSUM readout. Window of 3 PSUM
                # groups so the rank-1's wait on b_bf never stalls the PE.
                nmt = HIDL // 128
                WIN = 3
                pss = {}

                def emit_main(mt):
                    ps = ps_mm.tile([128, CHUNK], dt.float32, tag="mm")
                    for k in range(KT):
                        nc.tensor.matmul(ps, W['wf1'][k][:, mt*128:(mt+1)*128],
                                         xh2[k], start=(k == 0), stop=False)
                    pss[mt] = ps

                def emit_tail(mt):
                    ps = pss.pop(mt)
                    nc.tensor.matmul(ps, W['w1f1'][:, mt*128:(mt+1)*128], b_bf,
                                     start=False, stop=True)
                    hh = sqpool.tile([128, CHUNK], dt.bfloat16, name=f"hh{mt}",
                                     tag="sq")
                    nc.vector.tensor_mul(out=hh, in0=ps, in1=a_bc)
                    at = hpool.tile([128, CHUNK], dt.bfloat16, name=f"ga{mt}",
                                    tag=f"ga{mt}", bufs=1)
                    nc.scalar.activation(out=at, in_=hh, func=AF.Gelu)
                    a_sb.append(at)

                for mt in range(nmt):
                    emit_main(mt)
                    if mt >= WIN - 1:
                        emit_tail(mt - WIN + 1)
                for mt in range(nmt - WIN + 1, nmt):
                    emit_tail(mt)
            else:
                for mt in range(HIDL // 128):
                    ps = ps_mm.tile([128, CHUNK], dt.float32, tag="mm")
                    for k in range(KT):
                        nc.tensor.matmul(ps, W['wf1'][k][:, mt*128:(mt+1)*128],
                                         xh2[k],
                                         start=(k == 0), stop=(k == KT - 1))
                    at = hpool.tile([128, CHUNK], dt.bfloat16, name=f"ga{mt}",
                                    tag=f"ga{mt}", bufs=1)
                    if gelu_mode == "exact":
                        if bias_on["bf1"]:
                            nc.scalar.activation(out=at, in_=ps, func=AF.Gelu,
                                                 bias=bias_t["bf1"][:, mt:mt+1])
                        else:
                            nc.scalar.activation(out=at, in_=ps, func=AF.Gelu)
                    else:
                        assert not bias_on["bf1"]
                        sg = sqpool.tile([128, CHUNK], dt.bfloat16, name="sg",
                                         tag="sq")
                        nc.scalar.activation(out=sg, in_=ps, func=AF.Sigmoid,
                                             scale=1.702)
                        nc.vector.tensor_mul(out=at, in0=sg, in1=ps)
                    a_sb.append(at)
            stw = stpool.tile([128, KT * CHUNK], dt.bfloat16, name="f2st",
                              tag="stage", bufs=2)
            for mt in range(KT):
                ps = ps_mm.tile([128, CHUNK], dt.float32, tag="mm")
                for k in range(HIDL // 128):
                    nc.tensor.matmul(ps, W['wf2'][k][:, mt*128:(mt+1)*128], a_sb[k],
                                     start=(k == 0), stop=(k == HIDL // 128 - 1))
                st = stw[:, mt*CHUNK:(mt+1)*CHUNK]
                if bias_on["bf2"]:
                    nc.scalar.activation(out=st, in_=ps, func=AF.Identity,
                                         bias=bias_t["bf2"][:, mt:mt+1])
                else:
                    nc.scalar.activation(out=st, in_=ps, func=AF.Copy)
            return stw

        # Two-chunk software pipeline: each AllReduce is issued before ~80us of
        # independent compute from the other chunk, so its latency (plus the
        # serial residual->LN chain behind it) hides completely.
        pending = [None] * NCHUNK      # previous layer's fc2 AllReduces
        for li in range(L * loop_mult):
            l = li % L
            W = load_weights(l)
            kt_store, v_store = {}, {}
            ar_pr = [None] * NCHUNK
            up0 = up1 = None
            if pending[0] is not None:
                up0 = load_upd(pending[0], "f2p0")
            xh0 = layer_norm(f"ln1_{l}_0", 0, up0)
            qt2_0 = qkv_block(W, 0, xh0, kt_store, v_store)
            if interleave:
                ot0 = attn_block(0, qt2_0, kt_store, v_store, heads=[0, 1])
                if pending[1] is not None:
                    up1 = load_upd(pending[1], "f2p1")
                xh1 = layer_norm(f"ln1_{l}_1", 1, up1)
                qt2_1 = []
                filler = qkv_gen(W, 1, xh1, kt_store, v_store, qt2_1)
                attn_block(0, qt2_0, kt_store, v_store, ot_sb=ot0, heads=[2, 3],
                           filler=filler, fill_per_head=4)
                for _ in filler:
                    pass
                ar_pr[0] = all_reduce(proj_block(W, 0, ot0), f"pr_{l}_0")
            else:
                ot0 = attn_block(0, qt2_0, kt_store, v_store)
                ar_pr[0] = all_reduce(proj_block(W, 0, ot0), f"pr_{l}_0")
                if pending[1] is not None:
                    up1 = load_upd(pending[1], "f2p1")
                xh1 = layer_norm(f"ln1_{l}_1", 1, up1)
                qt2_1 = qkv_block(W, 1, xh1, kt_store, v_store)
            pending = [None] * NCHUNK
            ot1 = attn_block(1, qt2_1, kt_store, v_store)
            ar_pr[1] = all_reduce(proj_block(W, 1, ot1), f"pr_{l}_1")
            for c in range(NCHUNK):
                up = load_upd(ar_pr[c], f"pr{c}")
                if bias_on["bf1"] or gelu_mode != "exact":
                    xh2 = layer_norm(f"ln2_{l}_{c}", c, up, nxt=AF.Gelu)
                    stage2 = ffn_block(W, c, xh2)
                else:
                    xh2, a_bc2, b_bf2 = layer_norm(f"ln2_{l}_{c}", c, up,
                                                   nxt=AF.Gelu, raw=True)
                    stage2 = ffn_block(W, c, xh2, a_bc2, b_bf2)
                pending[c] = all_reduce(stage2, f"f2_{l}_{c}")
        # Epilogue: fold the last AllReduce into the fp32 output copy directly
        # (one less bf16 rounding on the final residual add).
        fin_up = [load_upd(pending[c], f"fin{c}") for c in range(NCHUNK)]
        for k in range(KT):
            of = stpool.tile([128, N], dt.float32, name=f"of{k}", tag="ofin", bufs=2)
            for c in range(NCHUNK):
                ts = slice(c * CHUNK, (c + 1) * CHUNK)
                nc.vector.tensor_add(out=of[:, ts], in0=xt[k][:, ts],
                                     in1=fin_up[c][:, k*CHUNK:(k+1)*CHUNK])
            nc.sync.dma_start(out=out_p[k], in_=of)

    if not nc.is_finalized():
        nc.finalize()
    return nc


def _prep_core_inputs(inputs, folded):
    """Per-core in_maps (host-side sharding + layout + bf16 cast)."""
    x = np.asarray(inputs['x'], np.float32)
    masks = _make_masks()
    scale = np.float32(D ** -0.5)

    per_core = []
    bias_on = {k: False for k in ("bqk", "bv", "bpr", "bf1", "bf2")}
    shard_cache = {}
    for cid in range(NCORES):
        r, b = cid % TP, cid // TP
        if r not in shard_cache:
            wqk, wv, wpr, wf1, wf2, w1f1 = [], [], [], [], [], []
            bqk, bv, bpr, bf1, bf2 = [], [], [], [], []
            for l in range(L):
                F = folded[l]
                Wq = F['Wqkv_t'][:, r*CL:(r+1)*CL] * scale
                Wk = F['Wqkv_t'][:, C + r*CL: C + (r+1)*CL]
                Wv = F['Wqkv_t'][:, 2*C + r*CL: 2*C + (r+1)*CL]
                wqk.append(np.concatenate([Wq, Wk], axis=1).reshape(KT, 128, 2*CL))
                wv.append(Wv.reshape(KT, 128, CL))
                wpr.append(F['Wproj_t'][r*CL:(r+1)*CL, :].reshape(CL//128, 128, C))
                wf1.append(F['Wfc1_t'][:, r*HIDL:(r+1)*HIDL].reshape(KT, 128, HIDL))
                w1f1.append(F['Wfc1_t'][:, r*HIDL:(r+1)*HIDL].sum(axis=0)
                            .reshape(1, HIDL))
                wf2.append(F['Wfc2_t'][r*HIDL:(r+1)*HIDL, :].reshape(HIDL//128, 128, C))
                bq = F['bqkv'][r*CL:(r+1)*CL] * scale
                bk = F['bqkv'][C + r*CL: C + (r+1)*CL]
                bqk.append(np.concatenate([bq, bk]).reshape(4, 128).T)
                bv.append(np.broadcast_to(
                    F['bqkv'][2*C + r*CL: 2*C + (r+1)*CL], (128, CL)).copy())
                bpr.append(F['bproj'].reshape(KT, 128).T / TP)
                bf1.append(F['bfc1'][r*HIDL:(r+1)*HIDL].reshape(KT, 128).T)
                bf2.append(F['bfc2'].reshape(KT, 128).T / TP)
            shard = dict(
                wqk=np.stack(wqk).astype(BF16), wv=np.stack(wv).astype(BF16),
                wpr=np.stack(wpr).astype(BF16), wf1=np.stack(wf1).astype(BF16),
                wf2=np.stack(wf2).astype(BF16),
                w1f1=np.stack(w1f1).astype(BF16),
                bqk=np.stack(bqk).astype(np.float32), bv=np.stack(bv).astype(np.float32),
                bpr=np.stack(bpr).astype(np.float32), bf1=np.stack(bf1).astype(np.float32),
                bf2=np.stack(bf2).astype(np.float32))
            shard_cache[r] = shard
        shard = shard_cache[r]
        m = dict(shard)
        m['xT'] = np.ascontiguousarray(x[b].T).reshape(KT, 128, N).astype(BF16)
        m['masks'] = masks
        per_core.append(m)

    for nm in bias_on:
        bias_on[nm] = any(bool(np.abs(m[nm]).max() > 0) for m in per_core)
    for m in per_core:
        for nm in list(m):
            if nm in bias_on and not bias_on[nm]:
                del m[nm]
    return per_core, bias_on


LAST_RESULT = None


def kernel(**inputs):
    global LAST_RESULT
    from concourse.bass_utils import run_bass_kernel_spmd
    folded = _fold_weights(inputs)
    in_maps, bias_on = _prep_core_inputs(inputs, folded)
    nc = build_program(bias_on)
    res = run_bass_kernel_spmd(nc, in_maps, core_ids=list(range(NCORES)))
    LAST_RESULT = res
    outs = []
    for b in range(B):
        o = res.results[b * TP]["out"].reshape(C, N).T    # [tokens, C]
        outs.append(o)
    return np.stack(outs).astype(np.float32)


if __name__ == "__main__":
    import reference
    inp = reference.setup_inputs()
    out = kernel(**{k: np.asarray(v) for k, v in inp.items()})
    exp = np.asarray(reference.reference(**inp))
    err = np.abs(out - exp).max() / np.abs(exp).max()
    print("Relative error:", err)

